# revision 17
# baseline (speedup 1.0000x reference)
"""Multi-head attention (B=8, N=1024, C=1024, H=16) on 8 TRN2 NeuronCores.

Data-parallel over batch: core b computes batch element b end-to-end; no
collectives. All matmuls bf16 with fp32 PSUM accumulation.

Structure:
  scores  row-tiled 64x128 matmul pairs: head A contracts over partitions
          0:64 (its d-rows of kT/qT), head B over 64:128, concurrently on
          the two row-halves of the PE array -> 2x scores throughput vs a
          zero-padded K=128 scheme, and q needs no zero-padded copy.
  exp     one ACTIVATE per j-tile over the 2-bank [headA|headB] PSUM pair,
          bf16 out straight into the AV-ready pT layout. ACT only does exp.
  AV+Z    col-tiled (128,64) matmul pairs: AV for heads A/B lands on
          partitions 0:64 / 64:128 of one PSUM bank, and ones-weight Z
          sums land on the SAME partitions of a second bank, so
          normalization is a single aligned reciprocal_approx_fast plus one
          tensor_mul (fused with the bf16 downcast) on DVE. ACT does no
          normalization work at all.
  proj    split into two ct-half passes (pairs 0-3 / 4-7) staged through an
          SBUF fp32 accumulator, so the first half runs as soon as pairs
          0-3 of its i-block are normalized instead of waiting for all 8.
  sched   explicit software pipeline: per window w, scores(block w) +
          AV/norm(block w-1) + a "filler" chunk of qkv/proj chains sized so
          neither PE nor ACT starves. qkv runs ct-major across 8 PSUM banks
          at startup so matmuls begin as soon as the first DMA chunks land;
          DMA issue order is prioritized (x + first weight quarters first).
"""

import numpy as np
import ml_dtypes

import concourse.bass as bass
import concourse.tile as tile
import concourse.tile_utils as tile_utils
from concourse import bacc, mybir, bass_utils

tile_utils.max_sbuf_usage = 208 * 1024  # stale 192KiB cap; cayman has 208 usable

# The scheduler's CoreSim cost model under-estimates ACTIVATE by ~8% vs HW
# (measured 1114ns for a 2-bank exp vs ~1030 modeled). Since the attention
# pipeline is exp-paced, that makes the static PE queue order place too few
# filler matmuls between successive score pairs, and the in-order queue then
# stalls ~200ns per exp period (head-of-line). Nudge the modeled ACT clock so
# the schedule spaces score pairs to the real exp rate. Scheduling-only: no
# effect on emitted instructions or numerics.
import concourse.hw_specs as _hw_specs

_hw_specs.TRN2Spec.CYCLE_T[mybir.EngineType.Activation] = 1e9 / 1.02e9

N = 1024   # sequence length
C = 1024   # model dim
H = 16     # heads
D = 64     # head dim
CT = 8     # 128-row tiles of c (contraction dim)
NT = 8     # 128-row tiles of n
NB = 2     # 512-wide blocks of n
PAIRS = 8

BF16 = mybir.dt.bfloat16
F32 = mybir.dt.float32

_nc_cache = None


def build_nc():
    global _nc_cache
    if _nc_cache is not None:
        return _nc_cache

    nc = bacc.Bacc("TRN2", target_bir_lowering=False, debug=False, num_devices=8)

    x_d = nc.dram_tensor("x", [C, N], BF16, kind="ExternalInput").ap()
    qkv_w_d = nc.dram_tensor("qkv_w", [C, 3 * C], BF16, kind="ExternalInput").ap()
    proj_w_d = nc.dram_tensor("proj_w", [C, C], BF16, kind="ExternalInput").ap()
    proj_b_d = nc.dram_tensor("proj_b", [C], BF16, kind="ExternalInput").ap()
    out_d = nc.dram_tensor("out", [N, C], F32, kind="ExternalOutput").ap()

    Exp = mybir.ActivationFunctionType.Exp

    with tile.TileContext(nc) as tc:
        with tc.tile_pool(name="big", bufs=1) as big, \
             tc.tile_pool(name="wk", bufs=2) as wk, \
             tc.tile_pool(name="ps", bufs=2, space="PSUM") as ps:

            xT_s = [big.tile([128, 2, N], BF16, name=f"xT{i}", tag=f"x{i}")
                    for i in range(4)]
            # q/k weights split by column quarter (q0: pairs 0-1, q1: 2-3)
            # then half (pairs 4-7), so the first chains' slices land first
            qwq_s = [[[big.tile([128, 2, 256], BF16, name=f"qwq{i}_{s}_{q}",
                                tag=f"qwq{i}_{s}_{q}") for q in range(2)]
                      for s in range(2)] for i in range(4)]
            qwh_s = [[big.tile([128, 2, 512], BF16, name=f"qwh{i}_{s}",
                               tag=f"qwh{i}_{s}") for s in range(2)]
                     for i in range(4)]
            # v weights by column half (= pair group g)
            vw_s = [[big.tile([128, 2, 512], BF16, name=f"vw{i}_{g}",
                              tag=f"vw{i}_{g}") for g in range(2)]
                    for i in range(4)]
            proj_wT_s = big.tile([128, CT, C], BF16)
            qT_s = [big.tile([128, N], BF16, name=f"qT{p}", tag=f"qT{p}")
                    for p in range(PAIRS)]
            kT_s = [big.tile([128, N], BF16, name=f"kT{p}", tag=f"kT{p}")
                    for p in range(PAIRS)]
            # v in natural qkv output layout: per n-tile, (pair, head, d)
            von_s = [big.tile([128, NT, 512], BF16, name=f"von{g}",
                              tag=f"von{g}") for g in range(2)]
            ones_s = big.tile([128, 64], BF16)
            aT_s = [[big.tile([128, 512], BF16, name=f"aT{p}_{ib}",
                              tag=f"aT{p}_{ib}") for ib in range(NB)]
                    for p in range(PAIRS)]
            bias_s = big.tile([128, C], BF16)

            def xT(ct):
                return xT_s[ct // 2][:, ct % 2, :]

            def qkw(ct, s, p):
                # [128, 128] weight slice of section s (0=q, 1=k) for pair p
                if p < 4:
                    t = qwq_s[ct // 2][s][p // 2]
                    return t[:, ct % 2, (p % 2) * 128:(p % 2 + 1) * 128]
                t = qwh_s[ct // 2][s]
                return t[:, ct % 2, (p - 4) * 128:(p - 3) * 128]

            def vw(ct, g):
                return vw_s[ct // 2][g][:, ct % 2, :]

            nc.gpsimd.memset(ones_s, 1.0)

            # ---- DMA issue order = priority ----
            for i in range(4):
                for h in range(2):
                    r = slice(i * 256 + h * 128, i * 256 + (h + 1) * 128)
                    nc.sync.dma_start(out=xT_s[i][:, h, :], in_=x_d[r, :])
                    for s in range(2):
                        nc.sync.dma_start(
                            out=qwq_s[i][s][0][:, h, :],
                            in_=qkv_w_d[r, s * C:s * C + 256])
            for i in range(4):
                for h in range(2):
                    r = slice(i * 256 + h * 128, i * 256 + (h + 1) * 128)
                    nc.sync.dma_start(out=vw_s[i][0][:, h, :],
                                      in_=qkv_w_d[r, 2 * C:2 * C + 512])
            for i in range(4):
                for h in range(2):
                    r = slice(i * 256 + h * 128, i * 256 + (h + 1) * 128)
                    for s in range(2):
                        nc.sync.dma_start(
                            out=qwq_s[i][s][1][:, h, :],
                            in_=qkv_w_d[r, s * C + 256:s * C + 512])
            for i in range(4):
                for h in range(2):
                    r = slice(i * 256 + h * 128, i * 256 + (h + 1) * 128)
                    for s in range(2):
                        nc.sync.dma_start(
                            out=qwh_s[i][s][:, h, :],
                            in_=qkv_w_d[r, s * C + 512:(s + 1) * C])
            for i in range(4):
                for h in range(2):
                    r = slice(i * 256 + h * 128, i * 256 + (h + 1) * 128)
                    nc.sync.dma_start(out=vw_s[i][1][:, h, :],
                                      in_=qkv_w_d[r, 2 * C + 512:3 * C])
            for ct in range(CT):
                nc.sync.dma_start(
                    out=proj_wT_s[:, ct, :],
                    in_=proj_w_d[ct * 128:(ct + 1) * 128, :])
            bias_bcast = bass.AP(
                tensor=proj_b_d.tensor,
                offset=proj_b_d.offset,
                ap=[[0, 128], proj_b_d.ap[0]],
            )
            nc.gpsimd.dma_start(out=bias_s, in_=bias_bcast)

            # ---- qkv helpers ----
            def qk_drain(p, which, nb, acc):
                dst = (qT_s if which == 0 else kT_s)[p]
                nc.vector.tensor_copy(out=dst[:, nb * 512:(nb + 1) * 512],
                                      in_=acc)

            def qk_ctmajor(pairs):
                # 8 chains across all 8 PSUM banks (borrow every tag), issued
                # ct-major so matmuls start as soon as DMA chunk ct lands
                specs = [(p, which, nb) for p in pairs for which in (0, 1)
                         for nb in range(NB)]
                st = [ps.tile([128, 2, 512], F32, tag="s",
                              name=f"qkm{pairs[0]}_{j}") for j in range(2)]
                accs = [st[0][:, 0, :], st[0][:, 1, :],
                        st[1][:, 0, :], st[1][:, 1, :]]
                accs.append(ps.tile([128, 512], F32, tag="qp",
                                    name=f"qkm{pairs[0]}_4"))
                accs.append(ps.tile([128, 512], F32, tag="qp",
                                    name=f"qkm{pairs[0]}_5"))
                accs.append(ps.tile([128, 512], F32, tag="o",
                                    name=f"qkm{pairs[0]}_6"))
                accs.append(ps.tile([128, 512], F32, tag="o",
                                    name=f"qkm{pairs[0]}_7"))
                for ct in range(CT):
                    for (p, which, nb), acc in zip(specs, accs):
                        nc.tensor.matmul(
                            acc, qkw(ct, which, p),
                            xT(ct)[:, nb * 512:(nb + 1) * 512],
                            start=(ct == 0), stop=(ct == CT - 1),
                            skip_group_check=True)
                for (p, which, nb), acc in zip(specs, accs):
                    qk_drain(p, which, nb, acc)

            def v_drain(g, nt, acc):
                nc.vector.tensor_copy(out=von_s[g][:, nt, :], in_=acc)

            def v_ctmajor(g, nts):
                accs = [ps.tile([128, 512], F32, tag=t, name=f"vm{g}_{nt}")
                        for nt, t in zip(nts, ("qp", "qp", "o", "o"))]
                for ct in range(CT):
                    for nt, acc in zip(nts, accs):
                        nc.tensor.matmul(
                            acc, xT(ct)[:, nt * 128:(nt + 1) * 128],
                            vw(ct, g), start=(ct == 0), stop=(ct == CT - 1),
                            skip_group_check=True)
                for nt, acc in zip(nts, accs):
                    v_drain(g, nt, acc)

            def qk_filler(p):
                # 4 sequential chains on the qp tag (mid-kernel filler)
                def mk(which, nb):
                    def f():
                        acc = ps.tile([128, 512], F32, tag="qp",
                                      name=f"qkf{p}_{which}_{nb}")
                        for ct in range(CT):
                            nc.tensor.matmul(
                                acc, qkw(ct, which, p),
                                xT(ct)[:, nb * 512:(nb + 1) * 512],
                                start=(ct == 0), stop=(ct == CT - 1),
                                skip_group_check=True)
                        qk_drain(p, which, nb, acc)
                    return f
                return [mk(0, 0), mk(0, 1), mk(1, 0), mk(1, 1)]

            def v_filler(g):
                def mk(nt):
                    def f():
                        acc = ps.tile([128, 512], F32, tag="qp",
                                      name=f"vf{g}_{nt}")
                        for ct in range(CT):
                            nc.tensor.matmul(
                                acc, xT(ct)[:, nt * 128:(nt + 1) * 128],
                                vw(ct, g), start=(ct == 0),
                                stop=(ct == CT - 1), skip_group_check=True)
                        v_drain(g, nt, acc)
                    return f
                return [mk(nt) for nt in range(NT)]

            y_of = {}

            def proj_part(nt, half):
                # proj split into two ct-half passes so the first half can
                # run as soon as pairs 0-3 of its i-block are normalized:
                # part 0 stages acc+bias into y; part 1 adds its acc on top
                # and DMAs out.
                ib = nt // 4

                def mk(ob):
                    def f():
                        obs = slice(ob * 512, (ob + 1) * 512)
                        acc = ps.tile([128, 512], F32, tag="qp",
                                      name=f"pr{nt}_{ob}_{half}")
                        for ct in range(half * 4, half * 4 + 4):
                            nc.tensor.matmul(
                                acc,
                                aT_s[ct][ib][:, (nt % 4) * 128:(nt % 4 + 1) * 128],
                                proj_wT_s[:, ct, obs],
                                start=(ct == half * 4), stop=(ct == half * 4 + 3),
                                skip_group_check=True)
                        if half == 0 and ob == 0:
                            y_of[nt] = wk.tile([128, C], F32, tag="y", bufs=4,
                                               name=f"y{nt}")
                        y = y_of[nt]
                        if half == 0:
                            nc.vector.tensor_add(out=y[:, obs], in0=acc,
                                                 in1=bias_s[:, obs])
                        else:
                            nc.vector.tensor_add(out=y[:, obs], in0=acc,
                                                 in1=y[:, obs])
                            if ob == 1:
                                nc.sync.dma_start(
                                    out=out_d[nt * 128:(nt + 1) * 128, :],
                                    in_=y)
                    return f
                return [mk(0), mk(1)]

            # ---- attention ----
            pT_of = {}

            def scores_exp(p, ib):
                ibs = slice(ib * 512, (ib + 1) * 512)
                pT = wk.tile([128, 2, NT, 512], BF16, tag="pT",
                             name=f"pT{p}_{ib}")
                pT_of[(p, ib)] = pT
                for jt in range(NT):
                    js = slice(jt * 128, (jt + 1) * 128)
                    s2 = ps.tile([128, 2, 512], F32, tag="s",
                                 name=f"s{p}_{ib}_{jt}")
                    nc.tensor.matmul(s2[:, 0, :], kT_s[p][0:64, js],
                                     qT_s[p][0:64, ibs], start=True, stop=True)
                    nc.tensor.matmul(s2[:, 1, :], kT_s[p][64:128, js],
                                     qT_s[p][64:128, ibs], start=True,
                                     stop=True)
                    nc.scalar.activation(out=pT[:, :, jt, :], in_=s2,
                                         func=Exp, scale=0.125)

            def av_norm(p, ib):
                # Col-tiled (128,64) matmul pairs: AV for heads A/B land on
                # partitions 0:64 / 64:128 of one bank, and the ones-weight
                # Z sums land on the SAME partitions of a second bank, so
                # normalization is one aligned reciprocal + one mul.
                g, q4 = p // 4, p % 4
                base = q4 * 128
                pT = pT_of.pop((p, ib))
                psAV = ps.tile([128, 512], F32, tag="o", name=f"psAV{p}_{ib}")
                psZ = ps.tile([128, 512], F32, tag="o", name=f"psZ{p}_{ib}")
                for jt in range(NT):
                    nc.tensor.matmul(
                        psAV[0:64, :], von_s[g][:, jt, base:base + 64],
                        pT[:, 0, jt, :], start=(jt == 0), stop=(jt == NT - 1),
                        skip_group_check=True)
                    nc.tensor.matmul(
                        psAV[64:128, :], von_s[g][:, jt, base + 64:base + 128],
                        pT[:, 1, jt, :], start=(jt == 0), stop=(jt == NT - 1),
                        skip_group_check=True)
                    nc.tensor.matmul(
                        psZ[0:64, :], ones_s, pT[:, 0, jt, :],
                        start=(jt == 0), stop=(jt == NT - 1),
                        skip_group_check=True)
                    nc.tensor.matmul(
                        psZ[64:128, :], ones_s, pT[:, 1, jt, :],
                        start=(jt == 0), stop=(jt == NT - 1),
                        skip_group_check=True)
                rz = wk.tile([128, 512], F32, tag="rz", bufs=2,
                             name=f"rz{p}_{ib}")
                nc.vector.reciprocal_approx_fast(out=rz, in_=psZ)
                nc.vector.tensor_mul(out=aT_s[p][ib], in0=psAV, in1=rz)

            # ---- schedule ----
            BLOCKS = [(0, 0), (1, 0), (0, 1), (2, 0), (3, 0), (4, 0), (5, 0),
                      (1, 1), (6, 0), (7, 0), (2, 1), (3, 1), (4, 1), (5, 1),
                      (6, 1), (7, 1)]
            # producers (qkv chains) must be emitted BEFORE av_norm of the
            # window's prev block (which may consume them); proj consumes
            # av_norm's aT output so it must be emitted AFTER.
            PRE = {
                1: [lambda: v_ctmajor(0, (0, 1, 2, 3)),
                    lambda: v_ctmajor(0, (4, 5, 6, 7))],
                2: qk_filler(2),
                3: qk_filler(3),
                4: qk_filler(4) + qk_filler(5),
                5: v_filler(1),
                7: qk_filler(6),
                8: qk_filler(7),
            }
            POST = {
                6: proj_part(0, 0) + proj_part(1, 0),
                8: proj_part(2, 0),
                9: proj_part(3, 0),
                10: proj_part(0, 1) + proj_part(1, 1),
                11: proj_part(2, 1) + proj_part(3, 1),
                12: proj_part(4, 0) + proj_part(5, 0),
                13: proj_part(6, 0) + proj_part(7, 0),
            }

            qk_ctmajor((0, 1))
            for w, blk in enumerate(BLOCKS):
                scores_exp(*blk)
                for f in PRE.get(w, []):
                    f()
                if w > 0:
                    av_norm(*BLOCKS[w - 1])
                for f in POST.get(w, []):
                    f()
            av_norm(*BLOCKS[-1])
            for nt in (4, 5, 6, 7):
                for f in proj_part(nt, 1):
                    f()

    nc.finalize()
    _nc_cache = nc
    return nc


def kernel(x, qkv_w, proj_w, proj_b, trace=False):
    nc = build_nc()
    bf = ml_dtypes.bfloat16
    x = np.asarray(x, dtype=np.float32)
    qkv_wT = np.ascontiguousarray(np.asarray(qkv_w, dtype=np.float32).T).astype(bf)
    proj_wT = np.ascontiguousarray(np.asarray(proj_w, dtype=np.float32).T).astype(bf)
    proj_b = np.ascontiguousarray(np.asarray(proj_b, dtype=np.float32)).astype(bf)

    in_maps = []
    for b in range(8):
        in_maps.append({
            "x": np.ascontiguousarray(x[b].T).astype(bf),
            "qkv_w": qkv_wT,
            "proj_w": proj_wT,
            "proj_b": proj_b,
        })

    res = bass_utils.run_bass_kernel_spmd(
        nc, in_maps, core_ids=list(range(8)), trace=trace)
    out = np.stack([
        np.asarray(res.results[b]["out"], dtype=np.float32) for b in range(8)])
    if trace:
        return out, res
    return out


# revision 20
# speedup vs baseline: 1.1682x; 1.1682x over previous
"""Multi-head attention (B=8, N=1024, C=1024, H=16) on 8 TRN2 NeuronCores.

Data-parallel over batch: core b computes batch element b end-to-end; no
collectives. All matmuls bf16 with fp32 PSUM accumulation.

Structure:
  scores  row-tiled 64x128 matmul pairs: head A contracts over partitions
          0:64 (its d-rows of kT/qT), head B over 64:128, concurrently on
          the two row-halves of the PE array -> 2x scores throughput vs a
          zero-padded K=128 scheme, and q needs no zero-padded copy.
  exp     one ACTIVATE per j-tile over the 2-bank [headA|headB] PSUM pair,
          bf16 out straight into the AV-ready pT layout. ACT only does exp.
  AV+Z    col-tiled (128,64) matmul pairs: AV for heads A/B lands on
          partitions 0:64 / 64:128 of one PSUM bank, and ones-weight Z
          sums land on the SAME partitions of a second bank, so
          normalization is a single aligned reciprocal_approx_fast plus one
          tensor_mul (fused with the bf16 downcast) on DVE. ACT does no
          normalization work at all.
  proj    split into two ct-half passes (pairs 0-3 / 4-7) staged through an
          SBUF fp32 accumulator, so the first half runs as soon as pairs
          0-3 of its i-block are normalized instead of waiting for all 8.
  sched   explicit software pipeline: per window w, scores(block w) +
          AV/norm(block w-1) + a "filler" chunk of qkv/proj chains sized so
          neither PE nor ACT starves. qkv runs ct-major across 8 PSUM banks
          at startup so matmuls begin as soon as the first DMA chunks land;
          DMA issue order is prioritized (x + first weight quarters first).
"""

import numpy as np
import ml_dtypes

import concourse.bass as bass
import concourse.tile as tile
import concourse.tile_utils as tile_utils
from concourse import bacc, mybir, bass_utils

tile_utils.max_sbuf_usage = 208 * 1024  # stale 192KiB cap; cayman has 208 usable

# The scheduler's CoreSim cost model under-estimates ACTIVATE by ~8% vs HW
# (measured 1114ns for a 2-bank exp vs ~1030 modeled). Since the attention
# pipeline is exp-paced, that makes the static PE queue order place too few
# filler matmuls between successive score pairs, and the in-order queue then
# stalls ~200ns per exp period (head-of-line). Nudge the modeled ACT clock so
# the schedule spaces score pairs to the real exp rate. Scheduling-only: no
# effect on emitted instructions or numerics.
import concourse.hw_specs as _hw_specs

_hw_specs.TRN2Spec.CYCLE_T[mybir.EngineType.Activation] = 1e9 / 1.02e9

N = 1024   # sequence length
C = 1024   # model dim
H = 16     # heads
D = 64     # head dim
CT = 8     # 128-row tiles of c (contraction dim)
NT = 8     # 128-row tiles of n
NB = 2     # 512-wide blocks of n
PAIRS = 8

BF16 = mybir.dt.bfloat16
F32 = mybir.dt.float32

_nc_cache = None


def build_nc():
    global _nc_cache
    if _nc_cache is not None:
        return _nc_cache

    nc = bacc.Bacc("TRN2", target_bir_lowering=False, debug=False, num_devices=8)

    x_d = nc.dram_tensor("x", [C, N], BF16, kind="ExternalInput").ap()
    qkv_w_d = nc.dram_tensor("qkv_w", [C, 3 * C], BF16, kind="ExternalInput").ap()
    proj_w_d = nc.dram_tensor("proj_w", [C, C], BF16, kind="ExternalInput").ap()
    proj_b_d = nc.dram_tensor("proj_b", [C], BF16, kind="ExternalInput").ap()
    out_d = nc.dram_tensor("out", [N, C], F32, kind="ExternalOutput").ap()

    Exp = mybir.ActivationFunctionType.Exp

    with tile.TileContext(nc) as tc:
        with tc.tile_pool(name="big", bufs=1) as big, \
             tc.tile_pool(name="wk", bufs=2) as wk, \
             tc.tile_pool(name="ps", bufs=2, space="PSUM") as ps:

            xT_s = [big.tile([128, 2, N], BF16, name=f"xT{i}", tag=f"x{i}")
                    for i in range(4)]
            # q/k weights split by column quarter (q0: pairs 0-1, q1: 2-3)
            # then half (pairs 4-7), so the first chains' slices land first
            qwq_s = [[[big.tile([128, 2, 256], BF16, name=f"qwq{i}_{s}_{q}",
                                tag=f"qwq{i}_{s}_{q}") for q in range(2)]
                      for s in range(2)] for i in range(4)]
            qwh_s = [[big.tile([128, 2, 512], BF16, name=f"qwh{i}_{s}",
                               tag=f"qwh{i}_{s}") for s in range(2)]
                     for i in range(4)]
            # v weights by column half (= pair group g)
            vw_s = [[big.tile([128, 2, 512], BF16, name=f"vw{i}_{g}",
                              tag=f"vw{i}_{g}") for g in range(2)]
                    for i in range(4)]
            proj_wT_s = big.tile([128, CT, C], BF16)
            qT_s = [big.tile([128, N], BF16, name=f"qT{p}", tag=f"qT{p}")
                    for p in range(PAIRS)]
            kT_s = [big.tile([128, N], BF16, name=f"kT{p}", tag=f"kT{p}")
                    for p in range(PAIRS)]
            # v in natural qkv output layout: per n-tile, (pair, head, d)
            von_s = [big.tile([128, NT, 512], BF16, name=f"von{g}",
                              tag=f"von{g}") for g in range(2)]
            ones_s = big.tile([128, 64], BF16)
            aT_s = [[big.tile([128, 512], BF16, name=f"aT{p}_{ib}",
                              tag=f"aT{p}_{ib}") for ib in range(NB)]
                    for p in range(PAIRS)]
            bias_s = big.tile([128, C], BF16)

            def xT(ct):
                return xT_s[ct // 2][:, ct % 2, :]

            def qkw(ct, s, p):
                # [128, 128] weight slice of section s (0=q, 1=k) for pair p
                if p < 4:
                    t = qwq_s[ct // 2][s][p // 2]
                    return t[:, ct % 2, (p % 2) * 128:(p % 2 + 1) * 128]
                t = qwh_s[ct // 2][s]
                return t[:, ct % 2, (p - 4) * 128:(p - 3) * 128]

            def vw(ct, g):
                return vw_s[ct // 2][g][:, ct % 2, :]

            nc.gpsimd.memset(ones_s, 1.0)

            # ---- DMA issue order = priority ----
            # Each dma_start costs ~690ns of issue time on its queue, and the
            # startup is issue-bound, not bandwidth-bound. Spread the first
            # wave across the SP/ACT/DVE/Pool queues (all idle at kernel
            # start) so the first qkv chains' inputs land ~3x sooner.
            for i in range(4):
                for h in range(2):
                    r = slice(i * 256 + h * 128, i * 256 + (h + 1) * 128)
                    nc.sync.dma_start(out=xT_s[i][:, h, :], in_=x_d[r, :])
                    nc.scalar.dma_start(
                        out=qwq_s[i][0][0][:, h, :],
                        in_=qkv_w_d[r, 0:256])
                    nc.vector.dma_start(
                        out=qwq_s[i][1][0][:, h, :],
                        in_=qkv_w_d[r, C:C + 256])
            for i in range(4):
                for h in range(2):
                    r = slice(i * 256 + h * 128, i * 256 + (h + 1) * 128)
                    nc.gpsimd.dma_start(out=vw_s[i][0][:, h, :],
                                        in_=qkv_w_d[r, 2 * C:2 * C + 512])
            for i in range(4):
                for h in range(2):
                    r = slice(i * 256 + h * 128, i * 256 + (h + 1) * 128)
                    for s in range(2):
                        nc.sync.dma_start(
                            out=qwq_s[i][s][1][:, h, :],
                            in_=qkv_w_d[r, s * C + 256:s * C + 512])
            for i in range(4):
                for h in range(2):
                    r = slice(i * 256 + h * 128, i * 256 + (h + 1) * 128)
                    for s in range(2):
                        nc.sync.dma_start(
                            out=qwh_s[i][s][:, h, :],
                            in_=qkv_w_d[r, s * C + 512:(s + 1) * C])
            for i in range(4):
                for h in range(2):
                    r = slice(i * 256 + h * 128, i * 256 + (h + 1) * 128)
                    nc.sync.dma_start(out=vw_s[i][1][:, h, :],
                                      in_=qkv_w_d[r, 2 * C + 512:3 * C])
            for ct in range(CT):
                nc.sync.dma_start(
                    out=proj_wT_s[:, ct, :],
                    in_=proj_w_d[ct * 128:(ct + 1) * 128, :])
            bias_bcast = bass.AP(
                tensor=proj_b_d.tensor,
                offset=proj_b_d.offset,
                ap=[[0, 128], proj_b_d.ap[0]],
            )
            nc.gpsimd.dma_start(out=bias_s, in_=bias_bcast)

            # ---- qkv helpers ----
            def qk_drain(p, which, nb, acc):
                dst = (qT_s if which == 0 else kT_s)[p]
                nc.vector.tensor_copy(out=dst[:, nb * 512:(nb + 1) * 512],
                                      in_=acc)

            def qk_ctmajor(pairs):
                # 8 chains across all 8 PSUM banks (borrow every tag), issued
                # ct-major so matmuls start as soon as DMA chunk ct lands
                specs = [(p, which, nb) for p in pairs for which in (0, 1)
                         for nb in range(NB)]
                st = [ps.tile([128, 2, 512], F32, tag="s",
                              name=f"qkm{pairs[0]}_{j}") for j in range(2)]
                accs = [st[0][:, 0, :], st[0][:, 1, :],
                        st[1][:, 0, :], st[1][:, 1, :]]
                accs.append(ps.tile([128, 512], F32, tag="qp",
                                    name=f"qkm{pairs[0]}_4"))
                accs.append(ps.tile([128, 512], F32, tag="qp",
                                    name=f"qkm{pairs[0]}_5"))
                accs.append(ps.tile([128, 512], F32, tag="o",
                                    name=f"qkm{pairs[0]}_6"))
                accs.append(ps.tile([128, 512], F32, tag="o",
                                    name=f"qkm{pairs[0]}_7"))
                for ct in range(CT):
                    for (p, which, nb), acc in zip(specs, accs):
                        nc.tensor.matmul(
                            acc, qkw(ct, which, p),
                            xT(ct)[:, nb * 512:(nb + 1) * 512],
                            start=(ct == 0), stop=(ct == CT - 1),
                            skip_group_check=True)
                for (p, which, nb), acc in zip(specs, accs):
                    qk_drain(p, which, nb, acc)

            def v_drain(g, nt, acc):
                nc.vector.tensor_copy(out=von_s[g][:, nt, :], in_=acc)

            def v_ctmajor(g, nts):
                accs = [ps.tile([128, 512], F32, tag=t, name=f"vm{g}_{nt}")
                        for nt, t in zip(nts, ("qp", "qp", "o", "o"))]
                for ct in range(CT):
                    for nt, acc in zip(nts, accs):
                        nc.tensor.matmul(
                            acc, xT(ct)[:, nt * 128:(nt + 1) * 128],
                            vw(ct, g), start=(ct == 0), stop=(ct == CT - 1),
                            skip_group_check=True)
                for nt, acc in zip(nts, accs):
                    v_drain(g, nt, acc)

            def qk_filler(p):
                # 4 sequential chains on the qp tag (mid-kernel filler)
                def mk(which, nb):
                    def f():
                        acc = ps.tile([128, 512], F32, tag="qp",
                                      name=f"qkf{p}_{which}_{nb}")
                        for ct in range(CT):
                            nc.tensor.matmul(
                                acc, qkw(ct, which, p),
                                xT(ct)[:, nb * 512:(nb + 1) * 512],
                                start=(ct == 0), stop=(ct == CT - 1),
                                skip_group_check=True)
                        qk_drain(p, which, nb, acc)
                    return f
                return [mk(0, 0), mk(0, 1), mk(1, 0), mk(1, 1)]

            def v_filler(g):
                def mk(nt):
                    def f():
                        acc = ps.tile([128, 512], F32, tag="qp",
                                      name=f"vf{g}_{nt}")
                        for ct in range(CT):
                            nc.tensor.matmul(
                                acc, xT(ct)[:, nt * 128:(nt + 1) * 128],
                                vw(ct, g), start=(ct == 0),
                                stop=(ct == CT - 1), skip_group_check=True)
                        v_drain(g, nt, acc)
                    return f
                return [mk(nt) for nt in range(NT)]

            y_of = {}

            def proj_span(nt, ct_lo, ct_hi, kind):
                # proj split into ct-span passes staged through an SBUF fp32
                # accumulator, so each span can run as soon as its pairs are
                # normalized. kind: "first" = y <- acc + bias, "mid" =
                # y += acc, "last" = y += acc then DMA out.
                ib = nt // 4

                def mk(ob):
                    def f():
                        obs = slice(ob * 512, (ob + 1) * 512)
                        acc = ps.tile([128, 512], F32, tag="qp",
                                      name=f"pr{nt}_{ob}_{ct_lo}")
                        for ct in range(ct_lo, ct_hi):
                            nc.tensor.matmul(
                                acc,
                                aT_s[ct][ib][:, (nt % 4) * 128:(nt % 4 + 1) * 128],
                                proj_wT_s[:, ct, obs],
                                start=(ct == ct_lo), stop=(ct == ct_hi - 1),
                                skip_group_check=True)
                        if kind == "first" and ob == 0:
                            y_of[nt] = wk.tile([128, C], F32, tag="y", bufs=4,
                                               name=f"y{nt}")
                        y = y_of[nt]
                        if kind == "first":
                            nc.vector.tensor_add(out=y[:, obs], in0=acc,
                                                 in1=bias_s[:, obs])
                        else:
                            nc.vector.tensor_add(out=y[:, obs], in0=acc,
                                                 in1=y[:, obs])
                            if kind == "last" and ob == 1:
                                nc.sync.dma_start(
                                    out=out_d[nt * 128:(nt + 1) * 128, :],
                                    in_=y)
                    return f
                return [mk(0), mk(1)]

            def proj_part(nt, half):
                if half == 0:
                    return proj_span(nt, 0, 4, "first")
                return proj_span(nt, 4, 8, "last")

            # ---- attention ----
            pT_of = {}

            def scores_exp(p, ib):
                ibs = slice(ib * 512, (ib + 1) * 512)
                pT = wk.tile([128, 2, NT, 512], BF16, tag="pT",
                             name=f"pT{p}_{ib}")
                pT_of[(p, ib)] = pT
                for jt in range(NT):
                    js = slice(jt * 128, (jt + 1) * 128)
                    s2 = ps.tile([128, 2, 512], F32, tag="s",
                                 name=f"s{p}_{ib}_{jt}")
                    nc.tensor.matmul(s2[:, 0, :], kT_s[p][0:64, js],
                                     qT_s[p][0:64, ibs], start=True, stop=True)
                    nc.tensor.matmul(s2[:, 1, :], kT_s[p][64:128, js],
                                     qT_s[p][64:128, ibs], start=True,
                                     stop=True)
                    nc.scalar.activation(out=pT[:, :, jt, :], in_=s2,
                                         func=Exp, scale=0.125)

            def av_norm(p, ib):
                # Col-tiled (128,64) matmul pairs: AV for heads A/B land on
                # partitions 0:64 / 64:128 of one bank, and the ones-weight
                # Z sums land on the SAME partitions of a second bank, so
                # normalization is one aligned reciprocal + one mul.
                g, q4 = p // 4, p % 4
                base = q4 * 128
                pT = pT_of.pop((p, ib))
                psAV = ps.tile([128, 512], F32, tag="o", name=f"psAV{p}_{ib}")
                psZ = ps.tile([128, 512], F32, tag="o", name=f"psZ{p}_{ib}")
                for jt in range(NT):
                    nc.tensor.matmul(
                        psAV[0:64, :], von_s[g][:, jt, base:base + 64],
                        pT[:, 0, jt, :], start=(jt == 0), stop=(jt == NT - 1),
                        skip_group_check=True)
                    nc.tensor.matmul(
                        psAV[64:128, :], von_s[g][:, jt, base + 64:base + 128],
                        pT[:, 1, jt, :], start=(jt == 0), stop=(jt == NT - 1),
                        skip_group_check=True)
                    nc.tensor.matmul(
                        psZ[0:64, :], ones_s, pT[:, 0, jt, :],
                        start=(jt == 0), stop=(jt == NT - 1),
                        skip_group_check=True)
                    nc.tensor.matmul(
                        psZ[64:128, :], ones_s, pT[:, 1, jt, :],
                        start=(jt == 0), stop=(jt == NT - 1),
                        skip_group_check=True)
                rz = wk.tile([128, 512], F32, tag="rz", bufs=2,
                             name=f"rz{p}_{ib}")
                nc.vector.reciprocal_approx_fast(out=rz, in_=psZ)
                nc.vector.tensor_mul(out=aT_s[p][ib], in0=psAV, in1=rz)

            # ---- schedule ----
            BLOCKS = [(0, 0), (1, 0), (0, 1), (2, 0), (3, 0), (4, 0), (5, 0),
                      (1, 1), (6, 0), (7, 0), (2, 1), (3, 1), (4, 1), (5, 1),
                      (6, 1), (7, 1)]
            # producers (qkv chains) must be emitted BEFORE av_norm of the
            # window's prev block (which may consume them); proj consumes
            # av_norm's aT output so it must be emitted AFTER.
            PRE = {
                1: [lambda: v_ctmajor(0, (0, 1, 2, 3)),
                    lambda: v_ctmajor(0, (4, 5, 6, 7))],
                2: qk_filler(2),
                3: qk_filler(3),
                4: qk_filler(4) + qk_filler(5),
                5: v_filler(1),
                7: qk_filler(6),
                8: qk_filler(7),
            }
            POST = {
                6: proj_part(0, 0) + proj_part(1, 0),
                8: proj_part(2, 0),
                9: proj_part(3, 0),
                10: proj_part(0, 1) + proj_part(1, 1),
                11: proj_part(2, 1) + proj_part(3, 1),
                12: proj_part(4, 0) + proj_part(5, 0),
                13: proj_part(6, 0) + proj_part(7, 0),
                # drain the ib1 projections as their pairs normalize, so the
                # post-loop tail is only the ct=7 contribution
                14: [f for nt in (4, 5, 6, 7)
                     for f in proj_span(nt, 4, 6, "mid")],
                15: [f for nt in (4, 5, 6, 7)
                     for f in proj_span(nt, 6, 7, "mid")],
            }

            qk_ctmajor((0, 1))
            for w, blk in enumerate(BLOCKS):
                scores_exp(*blk)
                for f in PRE.get(w, []):
                    f()
                if w > 0:
                    av_norm(*BLOCKS[w - 1])
                for f in POST.get(w, []):
                    f()
            av_norm(*BLOCKS[-1])
            for nt in (4, 5, 6, 7):
                for f in proj_span(nt, 7, 8, "last"):
                    f()

    nc.finalize()
    _nc_cache = nc
    return nc


def kernel(x, qkv_w, proj_w, proj_b, trace=False):
    nc = build_nc()
    bf = ml_dtypes.bfloat16
    x = np.asarray(x, dtype=np.float32)
    qkv_wT = np.ascontiguousarray(np.asarray(qkv_w, dtype=np.float32).T).astype(bf)
    proj_wT = np.ascontiguousarray(np.asarray(proj_w, dtype=np.float32).T).astype(bf)
    proj_b = np.ascontiguousarray(np.asarray(proj_b, dtype=np.float32)).astype(bf)

    in_maps = []
    for b in range(8):
        in_maps.append({
            "x": np.ascontiguousarray(x[b].T).astype(bf),
            "qkv_w": qkv_wT,
            "proj_w": proj_wT,
            "proj_b": proj_b,
        })

    res = bass_utils.run_bass_kernel_spmd(
        nc, in_maps, core_ids=list(range(8)), trace=trace)
    out = np.stack([
        np.asarray(res.results[b]["out"], dtype=np.float32) for b in range(8)])
    if trace:
        return out, res
    return out


# revision 23
# speedup vs baseline: 1.1752x; 1.0060x over previous
"""Multi-head attention (B=8, N=1024, C=1024, H=16) on 8 TRN2 NeuronCores.

Data-parallel over batch: core b computes batch element b end-to-end; no
collectives. All matmuls bf16 with fp32 PSUM accumulation.

Structure:
  scores  row-tiled 64x128 matmul pairs: head A contracts over partitions
          0:64 (its d-rows of kT/qT), head B over 64:128, concurrently on
          the two row-halves of the PE array -> 2x scores throughput vs a
          zero-padded K=128 scheme, and q needs no zero-padded copy.
  exp     one ACTIVATE per j-tile over the 2-bank [headA|headB] PSUM pair,
          bf16 out straight into the AV-ready pT layout. ACT only does exp.
  AV+Z    col-tiled (128,64) matmul pairs: AV for heads A/B lands on
          partitions 0:64 / 64:128 of one PSUM bank, and ones-weight Z
          sums land on the SAME partitions of a second bank, so
          normalization is a single aligned reciprocal_approx_fast plus one
          tensor_mul (fused with the bf16 downcast) on DVE. ACT does no
          normalization work at all.
  proj    split into two ct-half passes (pairs 0-3 / 4-7) staged through an
          SBUF fp32 accumulator, so the first half runs as soon as pairs
          0-3 of its i-block are normalized instead of waiting for all 8.
  sched   explicit software pipeline: per window w, scores(block w) +
          AV/norm(block w-1) + a "filler" chunk of qkv/proj chains sized so
          neither PE nor ACT starves. qkv runs ct-major across 8 PSUM banks
          at startup so matmuls begin as soon as the first DMA chunks land;
          DMA issue order is prioritized (x + first weight quarters first).
"""

import numpy as np
import ml_dtypes

import concourse.bass as bass
import concourse.tile as tile
import concourse.tile_utils as tile_utils
from concourse import bacc, mybir, bass_utils

tile_utils.max_sbuf_usage = 208 * 1024  # stale 192KiB cap; cayman has 208 usable

N = 1024   # sequence length
C = 1024   # model dim
H = 16     # heads
D = 64     # head dim
CT = 8     # 128-row tiles of c (contraction dim)
NT = 8     # 128-row tiles of n
NB = 2     # 512-wide blocks of n
PAIRS = 8

BF16 = mybir.dt.bfloat16
F32 = mybir.dt.float32

_nc_cache = None


def build_nc():
    global _nc_cache
    if _nc_cache is not None:
        return _nc_cache

    nc = bacc.Bacc("TRN2", target_bir_lowering=False, debug=False, num_devices=8)

    x_d = nc.dram_tensor("x", [C, N], BF16, kind="ExternalInput").ap()
    qkv_w_d = nc.dram_tensor("qkv_w", [C, 3 * C], BF16, kind="ExternalInput").ap()
    proj_w_d = nc.dram_tensor("proj_w", [C, C], BF16, kind="ExternalInput").ap()
    proj_b_d = nc.dram_tensor("proj_b", [C], BF16, kind="ExternalInput").ap()
    out_d = nc.dram_tensor("out", [N, C], F32, kind="ExternalOutput").ap()

    Exp = mybir.ActivationFunctionType.Exp

    with tile.TileContext(nc) as tc:
        with tc.tile_pool(name="big", bufs=1) as big, \
             tc.tile_pool(name="wk", bufs=2) as wk, \
             tc.tile_pool(name="ps", bufs=2, space="PSUM") as ps:

            xT_s = [big.tile([128, 2, N], BF16, name=f"xT{i}", tag=f"x{i}")
                    for i in range(4)]
            # q/k weights split by column quarter (q0: pairs 0-1, q1: 2-3)
            # then half (pairs 4-7), so the first chains' slices land first
            qwq_s = [[[big.tile([128, 2, 256], BF16, name=f"qwq{i}_{s}_{q}",
                                tag=f"qwq{i}_{s}_{q}") for q in range(2)]
                      for s in range(2)] for i in range(4)]
            qwh_s = [[big.tile([128, 2, 512], BF16, name=f"qwh{i}_{s}",
                               tag=f"qwh{i}_{s}") for s in range(2)]
                     for i in range(4)]
            # v weights by column half (= pair group g)
            vw_s = [[big.tile([128, 2, 512], BF16, name=f"vw{i}_{g}",
                              tag=f"vw{i}_{g}") for g in range(2)]
                    for i in range(4)]
            proj_wT_s = big.tile([128, CT, C], BF16)
            qT_s = [big.tile([128, N], BF16, name=f"qT{p}", tag=f"qT{p}")
                    for p in range(PAIRS)]
            kT_s = [big.tile([128, N], BF16, name=f"kT{p}", tag=f"kT{p}")
                    for p in range(PAIRS)]
            # v in natural qkv output layout: per n-tile, (pair, head, d)
            von_s = [big.tile([128, NT, 512], BF16, name=f"von{g}",
                              tag=f"von{g}") for g in range(2)]
            ones_s = big.tile([128, 64], BF16)
            aT_s = [[big.tile([128, 512], BF16, name=f"aT{p}_{ib}",
                              tag=f"aT{p}_{ib}") for ib in range(NB)]
                    for p in range(PAIRS)]
            bias_s = big.tile([128, C], BF16)

            def xT(ct):
                return xT_s[ct // 2][:, ct % 2, :]

            def qkw(ct, s, p):
                # [128, 128] weight slice of section s (0=q, 1=k) for pair p
                if p < 4:
                    t = qwq_s[ct // 2][s][p // 2]
                    return t[:, ct % 2, (p % 2) * 128:(p % 2 + 1) * 128]
                t = qwh_s[ct // 2][s]
                return t[:, ct % 2, (p - 4) * 128:(p - 3) * 128]

            def vw(ct, g):
                return vw_s[ct // 2][g][:, ct % 2, :]

            # ---- DMA issue order = priority ----
            # Each dma_start costs ~690ns of issue time on its queue, and the
            # startup is issue-bound, not bandwidth-bound. Spread the first
            # wave across the SP/ACT/Pool queues (the only DMA-capable ones,
            # all idle at kernel start) so the first qkv chains' inputs land
            # ~3x sooner.
            for i in range(4):
                for h in range(2):
                    r = slice(i * 256 + h * 128, i * 256 + (h + 1) * 128)
                    nc.sync.dma_start(out=xT_s[i][:, h, :], in_=x_d[r, :])
                    nc.scalar.dma_start(
                        out=qwq_s[i][0][0][:, h, :],
                        in_=qkv_w_d[r, 0:256])
                    nc.gpsimd.dma_start(
                        out=qwq_s[i][1][0][:, h, :],
                        in_=qkv_w_d[r, C:C + 256])
            nc.gpsimd.memset(ones_s, 1.0)
            for i in range(4):
                for h in range(2):
                    r = slice(i * 256 + h * 128, i * 256 + (h + 1) * 128)
                    nc.gpsimd.dma_start(out=vw_s[i][0][:, h, :],
                                        in_=qkv_w_d[r, 2 * C:2 * C + 512])
            for i in range(4):
                for h in range(2):
                    r = slice(i * 256 + h * 128, i * 256 + (h + 1) * 128)
                    for s in range(2):
                        nc.sync.dma_start(
                            out=qwq_s[i][s][1][:, h, :],
                            in_=qkv_w_d[r, s * C + 256:s * C + 512])
            for i in range(4):
                for h in range(2):
                    r = slice(i * 256 + h * 128, i * 256 + (h + 1) * 128)
                    for s in range(2):
                        nc.sync.dma_start(
                            out=qwh_s[i][s][:, h, :],
                            in_=qkv_w_d[r, s * C + 512:(s + 1) * C])
            for i in range(4):
                for h in range(2):
                    r = slice(i * 256 + h * 128, i * 256 + (h + 1) * 128)
                    nc.sync.dma_start(out=vw_s[i][1][:, h, :],
                                      in_=qkv_w_d[r, 2 * C + 512:3 * C])
            for ct in range(CT):
                nc.sync.dma_start(
                    out=proj_wT_s[:, ct, :],
                    in_=proj_w_d[ct * 128:(ct + 1) * 128, :])
            bias_bcast = bass.AP(
                tensor=proj_b_d.tensor,
                offset=proj_b_d.offset,
                ap=[[0, 128], proj_b_d.ap[0]],
            )
            nc.gpsimd.dma_start(out=bias_s, in_=bias_bcast)

            # ---- qkv helpers ----
            def qk_drain(p, which, nb, acc):
                dst = (qT_s if which == 0 else kT_s)[p]
                nc.vector.tensor_copy(out=dst[:, nb * 512:(nb + 1) * 512],
                                      in_=acc)

            def qk_ctmajor(pairs):
                # 8 chains across all 8 PSUM banks (borrow every tag), issued
                # ct-major so matmuls start as soon as DMA chunk ct lands
                specs = [(p, which, nb) for p in pairs for which in (0, 1)
                         for nb in range(NB)]
                st = [ps.tile([128, 2, 512], F32, tag="s",
                              name=f"qkm{pairs[0]}_{j}") for j in range(2)]
                accs = [st[0][:, 0, :], st[0][:, 1, :],
                        st[1][:, 0, :], st[1][:, 1, :]]
                accs.append(ps.tile([128, 512], F32, tag="qp",
                                    name=f"qkm{pairs[0]}_4"))
                accs.append(ps.tile([128, 512], F32, tag="qp",
                                    name=f"qkm{pairs[0]}_5"))
                accs.append(ps.tile([128, 512], F32, tag="o",
                                    name=f"qkm{pairs[0]}_6"))
                accs.append(ps.tile([128, 512], F32, tag="o",
                                    name=f"qkm{pairs[0]}_7"))
                for ct in range(CT):
                    for (p, which, nb), acc in zip(specs, accs):
                        nc.tensor.matmul(
                            acc, qkw(ct, which, p),
                            xT(ct)[:, nb * 512:(nb + 1) * 512],
                            start=(ct == 0), stop=(ct == CT - 1),
                            skip_group_check=True)
                for (p, which, nb), acc in zip(specs, accs):
                    qk_drain(p, which, nb, acc)

            def v_drain(g, nt, acc):
                nc.vector.tensor_copy(out=von_s[g][:, nt, :], in_=acc)

            def v_ctmajor(g, nts):
                accs = [ps.tile([128, 512], F32, tag=t, name=f"vm{g}_{nt}")
                        for nt, t in zip(nts, ("qp", "qp", "o", "o"))]
                for ct in range(CT):
                    for nt, acc in zip(nts, accs):
                        nc.tensor.matmul(
                            acc, xT(ct)[:, nt * 128:(nt + 1) * 128],
                            vw(ct, g), start=(ct == 0), stop=(ct == CT - 1),
                            skip_group_check=True)
                for nt, acc in zip(nts, accs):
                    v_drain(g, nt, acc)

            def qk_filler(p):
                # 4 sequential chains on the qp tag (mid-kernel filler)
                def mk(which, nb):
                    def f():
                        acc = ps.tile([128, 512], F32, tag="qp",
                                      name=f"qkf{p}_{which}_{nb}")
                        for ct in range(CT):
                            nc.tensor.matmul(
                                acc, qkw(ct, which, p),
                                xT(ct)[:, nb * 512:(nb + 1) * 512],
                                start=(ct == 0), stop=(ct == CT - 1),
                                skip_group_check=True)
                        qk_drain(p, which, nb, acc)
                    return f
                return [mk(0, 0), mk(0, 1), mk(1, 0), mk(1, 1)]

            def v_filler(g):
                def mk(nt):
                    def f():
                        acc = ps.tile([128, 512], F32, tag="qp",
                                      name=f"vf{g}_{nt}")
                        for ct in range(CT):
                            nc.tensor.matmul(
                                acc, xT(ct)[:, nt * 128:(nt + 1) * 128],
                                vw(ct, g), start=(ct == 0),
                                stop=(ct == CT - 1), skip_group_check=True)
                        v_drain(g, nt, acc)
                    return f
                return [mk(nt) for nt in range(NT)]

            y_of = {}

            def proj_span(nt, ct_lo, ct_hi, kind):
                # proj split into ct-span passes staged through an SBUF fp32
                # accumulator, so each span can run as soon as its pairs are
                # normalized. kind: "first" = y <- acc + bias, "mid" =
                # y += acc, "last" = y += acc then DMA out.
                ib = nt // 4

                def mk(ob):
                    def f():
                        obs = slice(ob * 512, (ob + 1) * 512)
                        acc = ps.tile([128, 512], F32, tag="qp",
                                      name=f"pr{nt}_{ob}_{ct_lo}")
                        for ct in range(ct_lo, ct_hi):
                            nc.tensor.matmul(
                                acc,
                                aT_s[ct][ib][:, (nt % 4) * 128:(nt % 4 + 1) * 128],
                                proj_wT_s[:, ct, obs],
                                start=(ct == ct_lo), stop=(ct == ct_hi - 1),
                                skip_group_check=True)
                        if kind == "first" and ob == 0:
                            y_of[nt] = wk.tile([128, C], F32, tag="y", bufs=4,
                                               name=f"y{nt}")
                        y = y_of[nt]
                        if kind == "first":
                            nc.vector.tensor_add(out=y[:, obs], in0=acc,
                                                 in1=bias_s[:, obs])
                        else:
                            nc.vector.tensor_add(out=y[:, obs], in0=acc,
                                                 in1=y[:, obs])
                            if kind == "last" and ob == 1:
                                nc.sync.dma_start(
                                    out=out_d[nt * 128:(nt + 1) * 128, :],
                                    in_=y)
                    return f
                return [mk(0), mk(1)]

            def proj_part(nt, half):
                if half == 0:
                    return proj_span(nt, 0, 4, "first")
                return proj_span(nt, 4, 8, "last")

            # ---- attention ----
            pT_of = {}

            def scores_exp(p, ib):
                ibs = slice(ib * 512, (ib + 1) * 512)
                pT = wk.tile([128, 2, NT, 512], BF16, tag="pT",
                             name=f"pT{p}_{ib}")
                pT_of[(p, ib)] = pT
                for jt in range(NT):
                    js = slice(jt * 128, (jt + 1) * 128)
                    s2 = ps.tile([128, 2, 512], F32, tag="s",
                                 name=f"s{p}_{ib}_{jt}")
                    nc.tensor.matmul(s2[:, 0, :], kT_s[p][0:64, js],
                                     qT_s[p][0:64, ibs], start=True, stop=True)
                    nc.tensor.matmul(s2[:, 1, :], kT_s[p][64:128, js],
                                     qT_s[p][64:128, ibs], start=True,
                                     stop=True)
                    nc.scalar.activation(out=pT[:, :, jt, :], in_=s2,
                                         func=Exp, scale=0.125)

            def av_norm(p, ib):
                # Col-tiled (128,64) matmul pairs: AV for heads A/B land on
                # partitions 0:64 / 64:128 of one bank, and the ones-weight
                # Z sums land on the SAME partitions of a second bank, so
                # normalization is one aligned reciprocal + one mul.
                g, q4 = p // 4, p % 4
                base = q4 * 128
                pT = pT_of.pop((p, ib))
                psAV = ps.tile([128, 512], F32, tag="o", name=f"psAV{p}_{ib}")
                psZ = ps.tile([128, 512], F32, tag="o", name=f"psZ{p}_{ib}")
                for jt in range(NT):
                    nc.tensor.matmul(
                        psAV[0:64, :], von_s[g][:, jt, base:base + 64],
                        pT[:, 0, jt, :], start=(jt == 0), stop=(jt == NT - 1),
                        skip_group_check=True)
                    nc.tensor.matmul(
                        psAV[64:128, :], von_s[g][:, jt, base + 64:base + 128],
                        pT[:, 1, jt, :], start=(jt == 0), stop=(jt == NT - 1),
                        skip_group_check=True)
                    nc.tensor.matmul(
                        psZ[0:64, :], ones_s, pT[:, 0, jt, :],
                        start=(jt == 0), stop=(jt == NT - 1),
                        skip_group_check=True)
                    nc.tensor.matmul(
                        psZ[64:128, :], ones_s, pT[:, 1, jt, :],
                        start=(jt == 0), stop=(jt == NT - 1),
                        skip_group_check=True)
                rz = wk.tile([128, 512], F32, tag="rz", bufs=2,
                             name=f"rz{p}_{ib}")
                nc.vector.reciprocal_approx_fast(out=rz, in_=psZ)
                nc.vector.tensor_mul(out=aT_s[p][ib], in0=psAV, in1=rz)

            # ---- schedule ----
            BLOCKS = [(0, 0), (1, 0), (0, 1), (2, 0), (3, 0), (4, 0), (5, 0),
                      (1, 1), (6, 0), (7, 0), (2, 1), (3, 1), (4, 1), (5, 1),
                      (6, 1), (7, 1)]
            # producers (qkv chains) must be emitted BEFORE av_norm of the
            # window's prev block (which may consume them); proj consumes
            # av_norm's aT output so it must be emitted AFTER.
            PRE = {
                1: [lambda: v_ctmajor(0, (0, 1, 2, 3)),
                    lambda: v_ctmajor(0, (4, 5, 6, 7))],
                2: qk_filler(2),
                3: qk_filler(3),
                4: qk_filler(4) + qk_filler(5),
                5: v_filler(1),
                7: qk_filler(6),
                8: qk_filler(7),
            }
            POST = {
                6: proj_part(0, 0) + proj_part(1, 0),
                8: proj_part(2, 0),
                9: proj_part(3, 0),
                10: proj_part(0, 1) + proj_part(1, 1),
                11: proj_part(2, 1) + proj_part(3, 1),
                12: proj_part(4, 0) + proj_part(5, 0),
                13: proj_part(6, 0) + proj_part(7, 0),
            }

            def proj_tail():
                # Final ct 4-7 projection for n-tiles 4-7. Scores are done,
                # so borrow the s banks too and run all 8 chains across 8
                # PSUM banks concurrently instead of rotating through 2.
                st = [ps.tile([128, 2, 512], F32, tag="s", name=f"prt{j}")
                      for j in range(2)]
                accs = [st[0][:, 0, :], st[0][:, 1, :],
                        st[1][:, 0, :], st[1][:, 1, :]]
                accs += [ps.tile([128, 512], F32, tag="qp", name=f"prt{4+j}")
                         for j in range(2)]
                accs += [ps.tile([128, 512], F32, tag="o", name=f"prt{6+j}")
                         for j in range(2)]
                units = [(nt, ob) for nt in (4, 5, 6, 7) for ob in range(2)]
                for (nt, ob), acc in zip(units, accs):
                    obs = slice(ob * 512, (ob + 1) * 512)
                    for ct in range(4, 8):
                        nc.tensor.matmul(
                            acc,
                            aT_s[ct][1][:, (nt % 4) * 128:(nt % 4 + 1) * 128],
                            proj_wT_s[:, ct, obs],
                            start=(ct == 4), stop=(ct == 7),
                            skip_group_check=True)
                for (nt, ob), acc in zip(units, accs):
                    obs = slice(ob * 512, (ob + 1) * 512)
                    y = y_of[nt]
                    nc.vector.tensor_add(out=y[:, obs], in0=acc,
                                         in1=y[:, obs])
                    if ob == 1:
                        nc.sync.dma_start(
                            out=out_d[nt * 128:(nt + 1) * 128, :], in_=y)

            qk_ctmajor((0, 1))
            for w, blk in enumerate(BLOCKS):
                scores_exp(*blk)
                for f in PRE.get(w, []):
                    f()
                if w > 0:
                    av_norm(*BLOCKS[w - 1])
                for f in POST.get(w, []):
                    f()
            av_norm(*BLOCKS[-1])
            proj_tail()

    nc.finalize()
    _nc_cache = nc
    return nc


def kernel(x, qkv_w, proj_w, proj_b, trace=False):
    nc = build_nc()
    bf = ml_dtypes.bfloat16
    x = np.asarray(x, dtype=np.float32)
    qkv_wT = np.ascontiguousarray(np.asarray(qkv_w, dtype=np.float32).T).astype(bf)
    proj_wT = np.ascontiguousarray(np.asarray(proj_w, dtype=np.float32).T).astype(bf)
    proj_b = np.ascontiguousarray(np.asarray(proj_b, dtype=np.float32)).astype(bf)

    in_maps = []
    for b in range(8):
        in_maps.append({
            "x": np.ascontiguousarray(x[b].T).astype(bf),
            "qkv_w": qkv_wT,
            "proj_w": proj_wT,
            "proj_b": proj_b,
        })

    res = bass_utils.run_bass_kernel_spmd(
        nc, in_maps, core_ids=list(range(8)), trace=trace)
    out = np.stack([
        np.asarray(res.results[b]["out"], dtype=np.float32) for b in range(8)])
    if trace:
        return out, res
    return out


# revision 25
# speedup vs baseline: 1.1797x; 1.0039x over previous
"""Multi-head attention (B=8, N=1024, C=1024, H=16) on 8 TRN2 NeuronCores.

Data-parallel over batch: core b computes batch element b end-to-end; no
collectives. All matmuls bf16 with fp32 PSUM accumulation.

Structure:
  scores  row-tiled 64x128 matmul pairs: head A contracts over partitions
          0:64 (its d-rows of kT/qT), head B over 64:128, concurrently on
          the two row-halves of the PE array -> 2x scores throughput vs a
          zero-padded K=128 scheme, and q needs no zero-padded copy.
  exp     one ACTIVATE per j-tile over the 2-bank [headA|headB] PSUM pair,
          bf16 out straight into the AV-ready pT layout. ACT only does exp.
  AV+Z    col-tiled (128,64) matmul pairs: AV for heads A/B lands on
          partitions 0:64 / 64:128 of one PSUM bank, and ones-weight Z
          sums land on the SAME partitions of a second bank, so
          normalization is a single aligned reciprocal_approx_fast plus one
          tensor_mul (fused with the bf16 downcast) on DVE. ACT does no
          normalization work at all.
  proj    split into two ct-half passes (pairs 0-3 / 4-7) staged through an
          SBUF fp32 accumulator, so the first half runs as soon as pairs
          0-3 of its i-block are normalized instead of waiting for all 8.
  sched   explicit software pipeline: per window w, scores(block w) +
          AV/norm(block w-1) + a "filler" chunk of qkv/proj chains sized so
          neither PE nor ACT starves. qkv runs ct-major across 8 PSUM banks
          at startup so matmuls begin as soon as the first DMA chunks land;
          DMA issue order is prioritized (x + first weight quarters first).
"""

import numpy as np
import ml_dtypes

import concourse.bass as bass
import concourse.tile as tile
import concourse.tile_utils as tile_utils
from concourse import bacc, mybir, bass_utils

tile_utils.max_sbuf_usage = 208 * 1024  # stale 192KiB cap; cayman has 208 usable

N = 1024   # sequence length
C = 1024   # model dim
H = 16     # heads
D = 64     # head dim
CT = 8     # 128-row tiles of c (contraction dim)
NT = 8     # 128-row tiles of n
NB = 2     # 512-wide blocks of n
PAIRS = 8

BF16 = mybir.dt.bfloat16
F32 = mybir.dt.float32

_nc_cache = None


def build_nc():
    global _nc_cache
    if _nc_cache is not None:
        return _nc_cache

    nc = bacc.Bacc("TRN2", target_bir_lowering=False, debug=False, num_devices=8)

    x_d = nc.dram_tensor("x", [C, N], BF16, kind="ExternalInput").ap()
    qkv_w_d = nc.dram_tensor("qkv_w", [C, 3 * C], BF16, kind="ExternalInput").ap()
    proj_w_d = nc.dram_tensor("proj_w", [C, C], BF16, kind="ExternalInput").ap()
    proj_b_d = nc.dram_tensor("proj_b", [C], BF16, kind="ExternalInput").ap()
    out_d = nc.dram_tensor("out", [N, C], F32, kind="ExternalOutput").ap()

    Exp = mybir.ActivationFunctionType.Exp

    with tile.TileContext(nc) as tc:
        with tc.tile_pool(name="big", bufs=1) as big, \
             tc.tile_pool(name="wk", bufs=2) as wk, \
             tc.tile_pool(name="ps", bufs=2, space="PSUM") as ps:

            xT_s = [big.tile([128, 2, N], BF16, name=f"xT{i}", tag=f"x{i}")
                    for i in range(4)]
            # q/k weights split by column quarter (q0: pairs 0-1, q1: 2-3)
            # then half (pairs 4-7), so the first chains' slices land first
            qwq_s = [[[big.tile([128, 2, 256], BF16, name=f"qwq{i}_{s}_{q}",
                                tag=f"qwq{i}_{s}_{q}") for q in range(2)]
                      for s in range(2)] for i in range(4)]
            qwh_s = [[big.tile([128, 2, 512], BF16, name=f"qwh{i}_{s}",
                               tag=f"qwh{i}_{s}") for s in range(2)]
                     for i in range(4)]
            # v weights by column half (= pair group g)
            vw_s = [[big.tile([128, 2, 512], BF16, name=f"vw{i}_{g}",
                              tag=f"vw{i}_{g}") for g in range(2)]
                    for i in range(4)]
            proj_wT_s = big.tile([128, CT, C], BF16)
            qT_s = [big.tile([128, N], BF16, name=f"qT{p}", tag=f"qT{p}")
                    for p in range(PAIRS)]
            kT_s = [big.tile([128, N], BF16, name=f"kT{p}", tag=f"kT{p}")
                    for p in range(PAIRS)]
            # v in natural qkv output layout: per n-tile, (pair, head, d)
            von_s = [big.tile([128, NT, 512], BF16, name=f"von{g}",
                              tag=f"von{g}") for g in range(2)]
            ones_s = big.tile([128, 64], BF16)
            aT_s = [[big.tile([128, 512], BF16, name=f"aT{p}_{ib}",
                              tag=f"aT{p}_{ib}") for ib in range(NB)]
                    for p in range(PAIRS)]
            bias_s = big.tile([128, C], BF16)

            def xT(ct):
                return xT_s[ct // 2][:, ct % 2, :]

            def qkw(ct, s, p):
                # [128, 128] weight slice of section s (0=q, 1=k) for pair p
                if p < 4:
                    t = qwq_s[ct // 2][s][p // 2]
                    return t[:, ct % 2, (p % 2) * 128:(p % 2 + 1) * 128]
                t = qwh_s[ct // 2][s]
                return t[:, ct % 2, (p - 4) * 128:(p - 3) * 128]

            def vw(ct, g):
                return vw_s[ct // 2][g][:, ct % 2, :]

            # ---- DMA issue order = priority ----
            # Each dma_start costs ~690ns of issue time on its queue, and the
            # startup is issue-bound, not bandwidth-bound. Spread the first
            # wave across the SP/ACT/Pool queues (the only DMA-capable ones,
            # all idle at kernel start) so the first qkv chains' inputs land
            # ~3x sooner.
            for i in range(4):
                for h in range(2):
                    r = slice(i * 256 + h * 128, i * 256 + (h + 1) * 128)
                    nc.sync.dma_start(out=xT_s[i][:, h, :], in_=x_d[r, :])
                    nc.scalar.dma_start(
                        out=qwq_s[i][0][0][:, h, :],
                        in_=qkv_w_d[r, 0:256])
                    nc.gpsimd.dma_start(
                        out=qwq_s[i][1][0][:, h, :],
                        in_=qkv_w_d[r, C:C + 256])
            for i in range(4):
                for h in range(2):
                    r = slice(i * 256 + h * 128, i * 256 + (h + 1) * 128)
                    nc.gpsimd.dma_start(out=vw_s[i][0][:, h, :],
                                        in_=qkv_w_d[r, 2 * C:2 * C + 512])
            nc.gpsimd.memset(ones_s, 1.0)
            for i in range(4):
                for h in range(2):
                    r = slice(i * 256 + h * 128, i * 256 + (h + 1) * 128)
                    for s in range(2):
                        nc.sync.dma_start(
                            out=qwq_s[i][s][1][:, h, :],
                            in_=qkv_w_d[r, s * C + 256:s * C + 512])
            for i in range(4):
                for h in range(2):
                    r = slice(i * 256 + h * 128, i * 256 + (h + 1) * 128)
                    for s in range(2):
                        nc.sync.dma_start(
                            out=qwh_s[i][s][:, h, :],
                            in_=qkv_w_d[r, s * C + 512:(s + 1) * C])
            for i in range(4):
                for h in range(2):
                    r = slice(i * 256 + h * 128, i * 256 + (h + 1) * 128)
                    nc.sync.dma_start(out=vw_s[i][1][:, h, :],
                                      in_=qkv_w_d[r, 2 * C + 512:3 * C])
            for ct in range(CT):
                nc.sync.dma_start(
                    out=proj_wT_s[:, ct, :],
                    in_=proj_w_d[ct * 128:(ct + 1) * 128, :])
            bias_bcast = bass.AP(
                tensor=proj_b_d.tensor,
                offset=proj_b_d.offset,
                ap=[[0, 128], proj_b_d.ap[0]],
            )
            nc.gpsimd.dma_start(out=bias_s, in_=bias_bcast)

            # ---- qkv helpers ----
            def qk_drain(p, which, nb, acc):
                dst = (qT_s if which == 0 else kT_s)[p]
                nc.vector.tensor_copy(out=dst[:, nb * 512:(nb + 1) * 512],
                                      in_=acc)

            def qk_ctmajor(pairs):
                # 8 chains across all 8 PSUM banks (borrow every tag), issued
                # ct-major so matmuls start as soon as DMA chunk ct lands
                specs = [(p, which, nb) for p in pairs for which in (0, 1)
                         for nb in range(NB)]
                st = [ps.tile([128, 2, 512], F32, tag="s",
                              name=f"qkm{pairs[0]}_{j}") for j in range(2)]
                accs = [st[0][:, 0, :], st[0][:, 1, :],
                        st[1][:, 0, :], st[1][:, 1, :]]
                accs.append(ps.tile([128, 512], F32, tag="qp",
                                    name=f"qkm{pairs[0]}_4"))
                accs.append(ps.tile([128, 512], F32, tag="qp",
                                    name=f"qkm{pairs[0]}_5"))
                accs.append(ps.tile([128, 512], F32, tag="o",
                                    name=f"qkm{pairs[0]}_6"))
                accs.append(ps.tile([128, 512], F32, tag="o",
                                    name=f"qkm{pairs[0]}_7"))
                for ct in range(CT):
                    for (p, which, nb), acc in zip(specs, accs):
                        nc.tensor.matmul(
                            acc, qkw(ct, which, p),
                            xT(ct)[:, nb * 512:(nb + 1) * 512],
                            start=(ct == 0), stop=(ct == CT - 1),
                            skip_group_check=True)
                for (p, which, nb), acc in zip(specs, accs):
                    qk_drain(p, which, nb, acc)

            def v_drain(g, nt, acc):
                nc.vector.tensor_copy(out=von_s[g][:, nt, :], in_=acc)

            def v_ctmajor(g, nts):
                accs = [ps.tile([128, 512], F32, tag=t, name=f"vm{g}_{nt}")
                        for nt, t in zip(nts, ("qp", "qp", "o", "o"))]
                for ct in range(CT):
                    for nt, acc in zip(nts, accs):
                        nc.tensor.matmul(
                            acc, xT(ct)[:, nt * 128:(nt + 1) * 128],
                            vw(ct, g), start=(ct == 0), stop=(ct == CT - 1),
                            skip_group_check=True)
                for nt, acc in zip(nts, accs):
                    v_drain(g, nt, acc)

            def qk_filler(p):
                # 4 sequential chains on the qp tag (mid-kernel filler)
                def mk(which, nb):
                    def f():
                        acc = ps.tile([128, 512], F32, tag="qp",
                                      name=f"qkf{p}_{which}_{nb}")
                        for ct in range(CT):
                            nc.tensor.matmul(
                                acc, qkw(ct, which, p),
                                xT(ct)[:, nb * 512:(nb + 1) * 512],
                                start=(ct == 0), stop=(ct == CT - 1),
                                skip_group_check=True)
                        qk_drain(p, which, nb, acc)
                    return f
                return [mk(0, 0), mk(0, 1), mk(1, 0), mk(1, 1)]

            def v_filler(g):
                def mk(nt):
                    def f():
                        acc = ps.tile([128, 512], F32, tag="qp",
                                      name=f"vf{g}_{nt}")
                        for ct in range(CT):
                            nc.tensor.matmul(
                                acc, xT(ct)[:, nt * 128:(nt + 1) * 128],
                                vw(ct, g), start=(ct == 0),
                                stop=(ct == CT - 1), skip_group_check=True)
                        v_drain(g, nt, acc)
                    return f
                return [mk(nt) for nt in range(NT)]

            y_of = {}

            def proj_span(nt, ct_lo, ct_hi, kind):
                # proj split into ct-span passes staged through an SBUF fp32
                # accumulator, so each span can run as soon as its pairs are
                # normalized. kind: "first" = y <- acc + bias, "mid" =
                # y += acc, "last" = y += acc then DMA out.
                ib = nt // 4

                def mk(ob):
                    def f():
                        obs = slice(ob * 512, (ob + 1) * 512)
                        acc = ps.tile([128, 512], F32, tag="qp",
                                      name=f"pr{nt}_{ob}_{ct_lo}")
                        for ct in range(ct_lo, ct_hi):
                            nc.tensor.matmul(
                                acc,
                                aT_s[ct][ib][:, (nt % 4) * 128:(nt % 4 + 1) * 128],
                                proj_wT_s[:, ct, obs],
                                start=(ct == ct_lo), stop=(ct == ct_hi - 1),
                                skip_group_check=True)
                        if kind == "first" and ob == 0:
                            y_of[nt] = wk.tile([128, C], F32, tag="y", bufs=4,
                                               name=f"y{nt}")
                        y = y_of[nt]
                        if kind == "first":
                            nc.vector.tensor_add(out=y[:, obs], in0=acc,
                                                 in1=bias_s[:, obs])
                        else:
                            nc.vector.tensor_add(out=y[:, obs], in0=acc,
                                                 in1=y[:, obs])
                            if kind == "last" and ob == 1:
                                nc.sync.dma_start(
                                    out=out_d[nt * 128:(nt + 1) * 128, :],
                                    in_=y)
                    return f
                return [mk(0), mk(1)]

            def proj_part(nt, half):
                if half == 0:
                    return proj_span(nt, 0, 4, "first")
                return proj_span(nt, 4, 8, "last")

            # ---- attention ----
            pT_of = {}

            def scores_exp(p, ib):
                ibs = slice(ib * 512, (ib + 1) * 512)
                pT = wk.tile([128, 2, NT, 512], BF16, tag="pT",
                             name=f"pT{p}_{ib}")
                pT_of[(p, ib)] = pT
                for jt in range(NT):
                    js = slice(jt * 128, (jt + 1) * 128)
                    s2 = ps.tile([128, 2, 512], F32, tag="s",
                                 name=f"s{p}_{ib}_{jt}")
                    nc.tensor.matmul(s2[:, 0, :], kT_s[p][0:64, js],
                                     qT_s[p][0:64, ibs], start=True, stop=True)
                    nc.tensor.matmul(s2[:, 1, :], kT_s[p][64:128, js],
                                     qT_s[p][64:128, ibs], start=True,
                                     stop=True)
                    nc.scalar.activation(out=pT[:, :, jt, :], in_=s2,
                                         func=Exp, scale=0.125)

            def av_norm(p, ib):
                # Col-tiled (128,64) matmul pairs: AV for heads A/B land on
                # partitions 0:64 / 64:128 of one bank, and the ones-weight
                # Z sums land on the SAME partitions of a second bank, so
                # normalization is one aligned reciprocal + one mul.
                g, q4 = p // 4, p % 4
                base = q4 * 128
                pT = pT_of.pop((p, ib))
                psAV = ps.tile([128, 512], F32, tag="o", name=f"psAV{p}_{ib}")
                psZ = ps.tile([128, 512], F32, tag="o", name=f"psZ{p}_{ib}")
                for jt in range(NT):
                    nc.tensor.matmul(
                        psAV[0:64, :], von_s[g][:, jt, base:base + 64],
                        pT[:, 0, jt, :], start=(jt == 0), stop=(jt == NT - 1),
                        skip_group_check=True)
                    nc.tensor.matmul(
                        psAV[64:128, :], von_s[g][:, jt, base + 64:base + 128],
                        pT[:, 1, jt, :], start=(jt == 0), stop=(jt == NT - 1),
                        skip_group_check=True)
                    nc.tensor.matmul(
                        psZ[0:64, :], ones_s, pT[:, 0, jt, :],
                        start=(jt == 0), stop=(jt == NT - 1),
                        skip_group_check=True)
                    nc.tensor.matmul(
                        psZ[64:128, :], ones_s, pT[:, 1, jt, :],
                        start=(jt == 0), stop=(jt == NT - 1),
                        skip_group_check=True)
                rz = wk.tile([128, 512], F32, tag="rz", bufs=2,
                             name=f"rz{p}_{ib}")
                nc.vector.reciprocal_approx_fast(out=rz, in_=psZ)
                nc.vector.tensor_mul(out=aT_s[p][ib], in0=psAV, in1=rz)

            # ---- schedule ----
            BLOCKS = [(0, 0), (1, 0), (0, 1), (1, 1), (2, 0), (3, 0),
                      (4, 0), (5, 0), (6, 0), (7, 0), (2, 1), (3, 1), (4, 1),
                      (5, 1), (6, 1), (7, 1)]
            # producers (qkv chains) must be emitted BEFORE av_norm of the
            # window's prev block (which may consume them); proj consumes
            # av_norm's aT output so it must be emitted AFTER.
            PRE = {
                1: [lambda: v_ctmajor(0, (0, 1, 2, 3)),
                    lambda: v_ctmajor(0, (4, 5, 6, 7))],
                2: qk_filler(2),
                3: qk_filler(3),
                4: qk_filler(4),
                5: qk_filler(5) + v_filler(1)[:8],
                6: v_filler(1)[8:],
                7: qk_filler(6),
                8: qk_filler(7),
            }
            POST = {
                7: proj_part(0, 0),
                8: proj_part(1, 0),
                9: proj_part(2, 0) + proj_part(3, 0),
                10: proj_part(0, 1) + proj_part(1, 1),
                11: proj_part(2, 1) + proj_part(3, 1),
                12: proj_part(4, 0) + proj_part(5, 0),
                13: proj_part(6, 0) + proj_part(7, 0),
            }

            def proj_tail():
                # Final ct 4-7 projection for n-tiles 4-7. Scores are done,
                # so borrow the s banks too and run all 8 chains across 8
                # PSUM banks concurrently instead of rotating through 2.
                st = [ps.tile([128, 2, 512], F32, tag="s", name=f"prt{j}")
                      for j in range(2)]
                accs = [st[0][:, 0, :], st[0][:, 1, :],
                        st[1][:, 0, :], st[1][:, 1, :]]
                accs += [ps.tile([128, 512], F32, tag="qp", name=f"prt{4+j}")
                         for j in range(2)]
                accs += [ps.tile([128, 512], F32, tag="o", name=f"prt{6+j}")
                         for j in range(2)]
                units = [(nt, ob) for nt in (4, 5, 6, 7) for ob in range(2)]
                for (nt, ob), acc in zip(units, accs):
                    obs = slice(ob * 512, (ob + 1) * 512)
                    for ct in range(4, 8):
                        nc.tensor.matmul(
                            acc,
                            aT_s[ct][1][:, (nt % 4) * 128:(nt % 4 + 1) * 128],
                            proj_wT_s[:, ct, obs],
                            start=(ct == 4), stop=(ct == 7),
                            skip_group_check=True)
                for (nt, ob), acc in zip(units, accs):
                    obs = slice(ob * 512, (ob + 1) * 512)
                    y = y_of[nt]
                    nc.vector.tensor_add(out=y[:, obs], in0=acc,
                                         in1=y[:, obs])
                    nc.sync.dma_start(
                        out=out_d[nt * 128:(nt + 1) * 128, obs],
                        in_=y[:, obs])

            qk_ctmajor((0, 1))
            for w, blk in enumerate(BLOCKS):
                scores_exp(*blk)
                for f in PRE.get(w, []):
                    f()
                if w > 0:
                    av_norm(*BLOCKS[w - 1])
                for f in POST.get(w, []):
                    f()
            av_norm(*BLOCKS[-1])
            proj_tail()

    nc.finalize()
    _nc_cache = nc
    return nc


def kernel(x, qkv_w, proj_w, proj_b, trace=False):
    nc = build_nc()
    bf = ml_dtypes.bfloat16
    x = np.asarray(x, dtype=np.float32)
    qkv_wT = np.ascontiguousarray(np.asarray(qkv_w, dtype=np.float32).T).astype(bf)
    proj_wT = np.ascontiguousarray(np.asarray(proj_w, dtype=np.float32).T).astype(bf)
    proj_b = np.ascontiguousarray(np.asarray(proj_b, dtype=np.float32)).astype(bf)

    in_maps = []
    for b in range(8):
        in_maps.append({
            "x": np.ascontiguousarray(x[b].T).astype(bf),
            "qkv_w": qkv_wT,
            "proj_w": proj_wT,
            "proj_b": proj_b,
        })

    res = bass_utils.run_bass_kernel_spmd(
        nc, in_maps, core_ids=list(range(8)), trace=trace)
    out = np.stack([
        np.asarray(res.results[b]["out"], dtype=np.float32) for b in range(8)])
    if trace:
        return out, res
    return out


# revision 27
# speedup vs baseline: 1.1860x; 1.0053x over previous
"""Multi-head attention (B=8, N=1024, C=1024, H=16) on 8 TRN2 NeuronCores.

Data-parallel over batch: core b computes batch element b end-to-end; no
collectives. All matmuls bf16 with fp32 PSUM accumulation.

Structure:
  scores  row-tiled 64x128 matmul pairs: head A contracts over partitions
          0:64 (its d-rows of kT/qT), head B over 64:128, concurrently on
          the two row-halves of the PE array -> 2x scores throughput vs a
          zero-padded K=128 scheme, and q needs no zero-padded copy.
  exp     one ACTIVATE per j-tile over the 2-bank [headA|headB] PSUM pair,
          bf16 out straight into the AV-ready pT layout. ACT only does exp.
  AV+Z    col-tiled (128,64) matmul pairs: AV for heads A/B lands on
          partitions 0:64 / 64:128 of one PSUM bank, and ones-weight Z
          sums land on the SAME partitions of a second bank, so
          normalization is a single aligned reciprocal_approx_fast plus one
          tensor_mul (fused with the bf16 downcast) on DVE. ACT does no
          normalization work at all.
  proj    split into two ct-half passes (pairs 0-3 / 4-7) staged through an
          SBUF fp32 accumulator, so the first half runs as soon as pairs
          0-3 of its i-block are normalized instead of waiting for all 8.
  sched   explicit software pipeline: per window w, scores(block w) +
          AV/norm(block w-1) + a "filler" chunk of qkv/proj chains sized so
          neither PE nor ACT starves. qkv runs ct-major across 8 PSUM banks
          at startup so matmuls begin as soon as the first DMA chunks land;
          DMA issue order is prioritized (x + first weight quarters first).
"""

import numpy as np
import ml_dtypes

import concourse.bass as bass
import concourse.tile as tile
import concourse.tile_utils as tile_utils
from concourse import bacc, mybir, bass_utils

tile_utils.max_sbuf_usage = 208 * 1024  # stale 192KiB cap; cayman has 208 usable

N = 1024   # sequence length
C = 1024   # model dim
H = 16     # heads
D = 64     # head dim
CT = 8     # 128-row tiles of c (contraction dim)
NT = 8     # 128-row tiles of n
NB = 2     # 512-wide blocks of n
PAIRS = 8

BF16 = mybir.dt.bfloat16
F32 = mybir.dt.float32

_nc_cache = None


def build_nc():
    global _nc_cache
    if _nc_cache is not None:
        return _nc_cache

    nc = bacc.Bacc("TRN2", target_bir_lowering=False, debug=False, num_devices=8)

    x_d = nc.dram_tensor("x", [C, N], BF16, kind="ExternalInput").ap()
    qkv_w_d = nc.dram_tensor("qkv_w", [C, 3 * C], BF16, kind="ExternalInput").ap()
    proj_w_d = nc.dram_tensor("proj_w", [C, C], BF16, kind="ExternalInput").ap()
    proj_b_d = nc.dram_tensor("proj_b", [C], BF16, kind="ExternalInput").ap()
    out_d = nc.dram_tensor("out", [N, C], F32, kind="ExternalOutput").ap()

    Exp = mybir.ActivationFunctionType.Exp

    with tile.TileContext(nc) as tc:
        with tc.tile_pool(name="big", bufs=1) as big, \
             tc.tile_pool(name="wk", bufs=2) as wk, \
             tc.tile_pool(name="ps", bufs=2, space="PSUM") as ps:

            xT_s = [big.tile([128, 2, N], BF16, name=f"xT{i}", tag=f"x{i}")
                    for i in range(4)]
            # q/k weights split by column quarter (q0: pairs 0-1, q1: 2-3)
            # then half (pairs 4-7), so the first chains' slices land first
            qwq_s = [[[big.tile([128, 2, 256], BF16, name=f"qwq{i}_{s}_{q}",
                                tag=f"qwq{i}_{s}_{q}") for q in range(2)]
                      for s in range(2)] for i in range(4)]
            qwh_s = [[big.tile([128, 2, 512], BF16, name=f"qwh{i}_{s}",
                               tag=f"qwh{i}_{s}") for s in range(2)]
                     for i in range(4)]
            # v weights by column half (= pair group g)
            vw_s = [[big.tile([128, 2, 512], BF16, name=f"vw{i}_{g}",
                              tag=f"vw{i}_{g}") for g in range(2)]
                    for i in range(4)]
            proj_wT_s = big.tile([128, CT, C], BF16)
            qT_s = [big.tile([128, N], BF16, name=f"qT{p}", tag=f"qT{p}")
                    for p in range(PAIRS)]
            kT_s = [big.tile([128, N], BF16, name=f"kT{p}", tag=f"kT{p}")
                    for p in range(PAIRS)]
            # v in natural qkv output layout: per n-tile, (pair, head, d)
            von_s = [big.tile([128, NT, 512], BF16, name=f"von{g}",
                              tag=f"von{g}") for g in range(2)]
            ones_s = big.tile([128, 64], BF16)
            aT_s = [[big.tile([128, 512], BF16, name=f"aT{p}_{ib}",
                              tag=f"aT{p}_{ib}") for ib in range(NB)]
                    for p in range(PAIRS)]
            bias_s = big.tile([128, C], BF16)

            def xT(ct):
                return xT_s[ct // 2][:, ct % 2, :]

            def qkw(ct, s, p):
                # [128, 128] weight slice of section s (0=q, 1=k) for pair p
                if p < 4:
                    t = qwq_s[ct // 2][s][p // 2]
                    return t[:, ct % 2, (p % 2) * 128:(p % 2 + 1) * 128]
                t = qwh_s[ct // 2][s]
                return t[:, ct % 2, (p - 4) * 128:(p - 3) * 128]

            def vw(ct, g):
                return vw_s[ct // 2][g][:, ct % 2, :]

            # ---- DMA issue order = priority ----
            # Each dma_start costs ~690ns of issue time on its queue, and the
            # startup is issue-bound, not bandwidth-bound. Spread the first
            # wave across the SP/ACT/Pool queues (the only DMA-capable ones,
            # all idle at kernel start) so the first qkv chains' inputs land
            # ~3x sooner.
            for i in range(4):
                for h in range(2):
                    r = slice(i * 256 + h * 128, i * 256 + (h + 1) * 128)
                    nc.sync.dma_start(out=xT_s[i][:, h, :], in_=x_d[r, :])
                    nc.scalar.dma_start(
                        out=qwq_s[i][0][0][:, h, :],
                        in_=qkv_w_d[r, 0:256])
                    nc.gpsimd.dma_start(
                        out=qwq_s[i][1][0][:, h, :],
                        in_=qkv_w_d[r, C:C + 256])
            for i in range(4):
                for h in range(2):
                    r = slice(i * 256 + h * 128, i * 256 + (h + 1) * 128)
                    nc.gpsimd.dma_start(out=vw_s[i][0][:, h, :],
                                        in_=qkv_w_d[r, 2 * C:2 * C + 512])
            nc.gpsimd.memset(ones_s, 1.0)
            for i in range(4):
                for h in range(2):
                    r = slice(i * 256 + h * 128, i * 256 + (h + 1) * 128)
                    for s in range(2):
                        nc.sync.dma_start(
                            out=qwq_s[i][s][1][:, h, :],
                            in_=qkv_w_d[r, s * C + 256:s * C + 512])
            for i in range(4):
                for h in range(2):
                    r = slice(i * 256 + h * 128, i * 256 + (h + 1) * 128)
                    for s in range(2):
                        nc.sync.dma_start(
                            out=qwh_s[i][s][:, h, :],
                            in_=qkv_w_d[r, s * C + 512:(s + 1) * C])
            for i in range(4):
                for h in range(2):
                    r = slice(i * 256 + h * 128, i * 256 + (h + 1) * 128)
                    nc.sync.dma_start(out=vw_s[i][1][:, h, :],
                                      in_=qkv_w_d[r, 2 * C + 512:3 * C])
            for ct in range(CT):
                nc.sync.dma_start(
                    out=proj_wT_s[:, ct, :],
                    in_=proj_w_d[ct * 128:(ct + 1) * 128, :])
            bias_bcast = bass.AP(
                tensor=proj_b_d.tensor,
                offset=proj_b_d.offset,
                ap=[[0, 128], proj_b_d.ap[0]],
            )
            nc.gpsimd.dma_start(out=bias_s, in_=bias_bcast)

            # ---- qkv helpers ----
            def qk_drain(p, which, nb, acc):
                dst = (qT_s if which == 0 else kT_s)[p]
                nc.vector.tensor_copy(out=dst[:, nb * 512:(nb + 1) * 512],
                                      in_=acc)

            def qk_ctmajor(pairs):
                # 8 chains across all 8 PSUM banks (borrow every tag), issued
                # ct-major so matmuls start as soon as DMA chunk ct lands
                specs = [(p, which, nb) for p in pairs for which in (0, 1)
                         for nb in range(NB)]
                st = [ps.tile([128, 2, 512], F32, tag="s",
                              name=f"qkm{pairs[0]}_{j}") for j in range(2)]
                accs = [st[0][:, 0, :], st[0][:, 1, :],
                        st[1][:, 0, :], st[1][:, 1, :]]
                accs.append(ps.tile([128, 512], F32, tag="qp",
                                    name=f"qkm{pairs[0]}_4"))
                accs.append(ps.tile([128, 512], F32, tag="qp",
                                    name=f"qkm{pairs[0]}_5"))
                accs.append(ps.tile([128, 512], F32, tag="o",
                                    name=f"qkm{pairs[0]}_6"))
                accs.append(ps.tile([128, 512], F32, tag="o",
                                    name=f"qkm{pairs[0]}_7"))
                for ct in range(CT):
                    for (p, which, nb), acc in zip(specs, accs):
                        nc.tensor.matmul(
                            acc, qkw(ct, which, p),
                            xT(ct)[:, nb * 512:(nb + 1) * 512],
                            start=(ct == 0), stop=(ct == CT - 1),
                            skip_group_check=True)
                for (p, which, nb), acc in zip(specs, accs):
                    qk_drain(p, which, nb, acc)

            def v_drain(g, nt, acc):
                nc.vector.tensor_copy(out=von_s[g][:, nt, :], in_=acc)

            def v_ctmajor(g, nts):
                accs = [ps.tile([128, 512], F32, tag=t, name=f"vm{g}_{nt}")
                        for nt, t in zip(nts, ("qp", "qp", "o", "o"))]
                for ct in range(CT):
                    for nt, acc in zip(nts, accs):
                        nc.tensor.matmul(
                            acc, xT(ct)[:, nt * 128:(nt + 1) * 128],
                            vw(ct, g), start=(ct == 0), stop=(ct == CT - 1),
                            skip_group_check=True)
                for nt, acc in zip(nts, accs):
                    v_drain(g, nt, acc)

            def qk_filler(p):
                # 4 sequential chains on the qp tag (mid-kernel filler)
                def mk(which, nb):
                    def f():
                        acc = ps.tile([128, 512], F32, tag="qp",
                                      name=f"qkf{p}_{which}_{nb}")
                        for ct in range(CT):
                            nc.tensor.matmul(
                                acc, qkw(ct, which, p),
                                xT(ct)[:, nb * 512:(nb + 1) * 512],
                                start=(ct == 0), stop=(ct == CT - 1),
                                skip_group_check=True)
                        qk_drain(p, which, nb, acc)
                    return f
                return [mk(0, 0), mk(0, 1), mk(1, 0), mk(1, 1)]

            def v_filler(g):
                def mk(nt):
                    def f():
                        acc = ps.tile([128, 512], F32, tag="qp",
                                      name=f"vf{g}_{nt}")
                        for ct in range(CT):
                            nc.tensor.matmul(
                                acc, xT(ct)[:, nt * 128:(nt + 1) * 128],
                                vw(ct, g), start=(ct == 0),
                                stop=(ct == CT - 1), skip_group_check=True)
                        v_drain(g, nt, acc)
                    return f
                return [mk(nt) for nt in range(NT)]

            y_of = {}

            def proj_span(nt, ct_lo, ct_hi, kind):
                # proj split into ct-span passes staged through an SBUF fp32
                # accumulator, so each span can run as soon as its pairs are
                # normalized. kind: "first" = y <- acc + bias, "mid" =
                # y += acc, "last" = y += acc then DMA out.
                ib = nt // 4

                def mk(ob):
                    def f():
                        obs = slice(ob * 512, (ob + 1) * 512)
                        acc = ps.tile([128, 512], F32, tag="qp",
                                      name=f"pr{nt}_{ob}_{ct_lo}")
                        for ct in range(ct_lo, ct_hi):
                            nc.tensor.matmul(
                                acc,
                                aT_s[ct][ib][:, (nt % 4) * 128:(nt % 4 + 1) * 128],
                                proj_wT_s[:, ct, obs],
                                start=(ct == ct_lo), stop=(ct == ct_hi - 1),
                                skip_group_check=True)
                        if kind == "first" and ob == 0:
                            y_of[nt] = wk.tile([128, C], F32, tag="y", bufs=4,
                                               name=f"y{nt}")
                        y = y_of[nt]
                        if kind == "first":
                            nc.vector.tensor_add(out=y[:, obs], in0=acc,
                                                 in1=bias_s[:, obs])
                        else:
                            nc.vector.tensor_add(out=y[:, obs], in0=acc,
                                                 in1=y[:, obs])
                            if kind == "last" and ob == 1:
                                nc.sync.dma_start(
                                    out=out_d[nt * 128:(nt + 1) * 128, :],
                                    in_=y)
                    return f
                return [mk(0), mk(1)]

            def proj_part(nt, half):
                if half == 0:
                    return proj_span(nt, 0, 4, "first")
                return proj_span(nt, 4, 8, "last")

            # ---- attention ----
            pT_of = {}

            def scores_exp(p, ib):
                ibs = slice(ib * 512, (ib + 1) * 512)
                pT = wk.tile([128, 2, NT, 512], BF16, tag="pT",
                             name=f"pT{p}_{ib}")
                pT_of[(p, ib)] = pT
                for jt in range(NT):
                    js = slice(jt * 128, (jt + 1) * 128)
                    s2 = ps.tile([128, 2, 512], F32, tag="s",
                                 name=f"s{p}_{ib}_{jt}")
                    nc.tensor.matmul(s2[:, 0, :], kT_s[p][0:64, js],
                                     qT_s[p][0:64, ibs], start=True, stop=True)
                    nc.tensor.matmul(s2[:, 1, :], kT_s[p][64:128, js],
                                     qT_s[p][64:128, ibs], start=True,
                                     stop=True)
                    nc.scalar.activation(out=pT[:, :, jt, :], in_=s2,
                                         func=Exp, scale=0.125)

            def av_norm(p, ib):
                # Col-tiled (128,64) matmul pairs: AV for heads A/B land on
                # partitions 0:64 / 64:128 of one bank, and the ones-weight
                # Z sums land on the SAME partitions of a second bank, so
                # normalization is one aligned reciprocal + one mul.
                g, q4 = p // 4, p % 4
                base = q4 * 128
                pT = pT_of.pop((p, ib))
                psAV = ps.tile([128, 512], F32, tag="o", name=f"psAV{p}_{ib}")
                psZ = ps.tile([128, 512], F32, tag="o", name=f"psZ{p}_{ib}")
                for jt in range(NT):
                    nc.tensor.matmul(
                        psAV[0:64, :], von_s[g][:, jt, base:base + 64],
                        pT[:, 0, jt, :], start=(jt == 0), stop=(jt == NT - 1),
                        skip_group_check=True)
                    nc.tensor.matmul(
                        psAV[64:128, :], von_s[g][:, jt, base + 64:base + 128],
                        pT[:, 1, jt, :], start=(jt == 0), stop=(jt == NT - 1),
                        skip_group_check=True)
                    nc.tensor.matmul(
                        psZ[0:64, :], ones_s, pT[:, 0, jt, :],
                        start=(jt == 0), stop=(jt == NT - 1),
                        skip_group_check=True)
                    nc.tensor.matmul(
                        psZ[64:128, :], ones_s, pT[:, 1, jt, :],
                        start=(jt == 0), stop=(jt == NT - 1),
                        skip_group_check=True)
                rz = wk.tile([128, 512], F32, tag="rz", bufs=2,
                             name=f"rz{p}_{ib}")
                nc.vector.reciprocal_approx_fast(out=rz, in_=psZ)
                nc.vector.tensor_mul(out=aT_s[p][ib], in0=psAV, in1=rz)

            # ---- schedule ----
            BLOCKS = [(0, 0), (1, 0), (0, 1), (1, 1), (2, 0), (3, 0),
                      (4, 0), (5, 0), (6, 0), (7, 0), (2, 1), (3, 1), (4, 1),
                      (5, 1), (6, 1), (7, 1)]
            # producers (qkv chains) must be emitted BEFORE av_norm of the
            # window's prev block (which may consume them); proj consumes
            # av_norm's aT output so it must be emitted AFTER.
            PRE = {
                1: [lambda: v_ctmajor(0, (0, 1, 2, 3)),
                    lambda: v_ctmajor(0, (4, 5, 6, 7))],
                2: qk_filler(2),
                3: qk_filler(3),
                4: qk_filler(4),
                5: qk_filler(5) + v_filler(1)[:8],
                6: v_filler(1)[8:],
                7: qk_filler(6),
                8: qk_filler(7),
            }
            POST = {
                7: proj_part(0, 0),
                8: proj_part(1, 0),
                9: proj_part(2, 0) + proj_part(3, 0),
                10: proj_part(0, 1) + proj_part(1, 1),
                11: proj_part(2, 1) + proj_part(3, 1),
                12: proj_part(4, 0) + proj_part(5, 0),
                13: proj_part(6, 0) + proj_part(7, 0),
            }

            def proj_tail():
                # Final ct 4-7 projection for n-tiles 4-7 across 8 banks.
                st = [ps.tile([128, 2, 512], F32, tag="s", name=f"prt{j}")
                      for j in range(2)]
                accs = [st[0][:, 0, :], st[0][:, 1, :],
                        st[1][:, 0, :], st[1][:, 1, :]]
                accs += [ps.tile([128, 512], F32, tag="qp", name=f"prt{4+j}")
                         for j in range(2)]
                accs += [ps.tile([128, 512], F32, tag="o", name=f"prt{6+j}")
                         for j in range(2)]
                units = [(nt, ob) for nt in (4, 5, 6, 7) for ob in range(2)]
                for (nt, ob), acc in zip(units, accs):
                    obs = slice(ob * 512, (ob + 1) * 512)
                    for ct in range(4, 8):
                        nc.tensor.matmul(
                            acc,
                            aT_s[ct][1][:, (nt % 4) * 128:(nt % 4 + 1) * 128],
                            proj_wT_s[:, ct, obs],
                            start=(ct == 4), stop=(ct == 7),
                            skip_group_check=True)
                for (nt, ob), acc in zip(units, accs):
                    obs = slice(ob * 512, (ob + 1) * 512)
                    y = y_of[nt]
                    nc.vector.tensor_add(out=y[:, obs], in0=acc,
                                         in1=y[:, obs])
                    nc.sync.dma_start(
                        out=out_d[nt * 128:(nt + 1) * 128, obs],
                        in_=y[:, obs])

            qk_ctmajor((0, 1))
            for w, blk in enumerate(BLOCKS):
                scores_exp(*blk)
                for f in PRE.get(w, []):
                    f()
                if w > 0:
                    av_norm(*BLOCKS[w - 1])
                for f in POST.get(w, []):
                    f()
            av_norm(*BLOCKS[-1])
            proj_tail()

    nc.finalize()
    _nc_cache = nc
    return nc


def kernel(x, qkv_w, proj_w, proj_b, trace=False):
    nc = build_nc()
    bf = ml_dtypes.bfloat16
    x = np.asarray(x, dtype=np.float32)
    qkv_wT = np.ascontiguousarray(np.asarray(qkv_w, dtype=np.float32).T).astype(bf)
    proj_wT = np.ascontiguousarray(np.asarray(proj_w, dtype=np.float32).T).astype(bf)
    proj_b = np.ascontiguousarray(np.asarray(proj_b, dtype=np.float32)).astype(bf)

    in_maps = []
    for b in range(8):
        in_maps.append({
            "x": np.ascontiguousarray(x[b].T).astype(bf),
            "qkv_w": qkv_wT,
            "proj_w": proj_wT,
            "proj_b": proj_b,
        })

    res = bass_utils.run_bass_kernel_spmd(
        nc, in_maps, core_ids=list(range(8)), trace=trace)
    out = np.stack([
        np.asarray(res.results[b]["out"], dtype=np.float32) for b in range(8)])
    if trace:
        return out, res
    return out
